# revision 1
# baseline (speedup 1.0000x reference)
"""BiLSTM-CRF forward loss on 8 Trainium2 NeuronCores.

Data-parallel: batch 64 -> 8 sequences per core. Each core runs
embedding gather -> BiLSTM(T=512,H=256) -> fc1(32)+relu -> fc2(4),
and outputs its emissions [4, T*8]. The tiny CRF dynamic program
(O(T*B*K^2), K=4) and the final mean run on host in numpy.
"""

import sys
for _p in ("/opt/trn_rl_repo", "/root/.axon_site/_ro/trn_rl_repo"):
    if _p not in sys.path:
        sys.path.insert(0, _p)
import numpy as np
from ml_dtypes import bfloat16

import concourse.bass as bass
import concourse.bacc as bacc
import concourse.mybir as mybir
from concourse.tile import TileContext
from concourse import bass_utils

B, T, E, H, V, K = 64, 512, 300, 256, 50000, 4
NCORES = 8
BC = B // NCORES          # 8 sequences per core
EP = 304                  # E padded to 304; row 300 = ones (bias trick)
G4H = 4 * H               # 1024
F32 = mybir.dt.float32
BF16 = mybir.dt.bfloat16
I32 = mybir.dt.int32
FP8 = mybir.dt.float8e4
AF = mybir.ActivationFunctionType
ALU = mybir.AluOpType


def build_bass(t_steps=T, bc=BC, parts="012f"):
    TOK = t_steps * bc
    nc = bacc.Bacc()

    # ---- DRAM parameters ----
    emb_aug = nc.dram_tensor("emb_aug", [V, EP], F32, kind="ExternalInput")
    toks = nc.dram_tensor("toks", [TOK, 1], I32, kind="ExternalInput")
    wxf = nc.dram_tensor("wxf", [EP, G4H], BF16, kind="ExternalInput")
    wxb = nc.dram_tensor("wxb", [EP, G4H], BF16, kind="ExternalInput")
    whf = nc.dram_tensor("whf", [H, G4H], BF16, kind="ExternalInput")
    whb = nc.dram_tensor("whb", [H, G4H], BF16, kind="ExternalInput")
    fc1w = nc.dram_tensor("fc1w", [2 * H, 32], BF16, kind="ExternalInput")
    fc1b = nc.dram_tensor("fc1b", [32, 1], F32, kind="ExternalInput")
    fc2w = nc.dram_tensor("fc2w", [32, K], BF16, kind="ExternalInput")
    fc2b = nc.dram_tensor("fc2b", [K, 1], F32, kind="ExternalInput")
    iden = nc.dram_tensor("iden", [128, 128], F32, kind="ExternalInput")
    out = nc.dram_tensor("out", [K, TOK], F32, kind="ExternalOutput")

    n_ttile = TOK // 128          # token tiles of 128
    n_n512 = TOK // 512           # 512-wide token chunks
    ek = [(0, 128), (128, 128), (256, 48)]   # E-chunks (rows of EP)

    with TileContext(nc) as tc:
        with tc.tile_pool(name="const", bufs=1) as constp, \
             tc.tile_pool(name="persist", bufs=1) as pp:
            # constants in SBUF
            id_sb = constp.tile([128, 128], F32, tag="iden")
            nc.sync.dma_start(id_sb[:], iden[:])
            wx_sb = {}
            for d, src in (("f", wxf), ("b", wxb)):
                for ki, (r0, rn) in enumerate(ek):
                    w = constp.tile([rn, G4H], BF16, tag=f"wx{d}{ki}")
                    nc.sync.dma_start(w[:], src[r0:r0 + rn, :])
                    wx_sb[(d, ki)] = w
            wh_sb = {}
            for d, src in (("f", whf), ("b", whb)):
                for ki in range(2):
                    w = constp.tile([128, G4H], BF16, tag=f"wh{d}{ki}")
                    nc.sync.dma_start(w[:], src[ki * 128:(ki + 1) * 128, :])
                    wh_sb[(d, ki)] = w
            fc1w_sb = []
            for ki in range(4):
                w = constp.tile([128, 32], BF16, tag=f"fc1w{ki}")
                nc.sync.dma_start(w[:], fc1w[ki * 128:(ki + 1) * 128, :])
                fc1w_sb.append(w)
            fc2w_sb = constp.tile([32, K], BF16, tag="fc2w")
            nc.sync.dma_start(fc2w_sb[:], fc2w[:])
            fc1b_sb = constp.tile([32, 1], F32, tag="fc1b")
            nc.sync.dma_start(fc1b_sb[:], fc1b[:])
            fc2b_sb = constp.tile([K, 1], F32, tag="fc2b")
            nc.sync.dma_start(fc2b_sb[:], fc2b[:])

            # persistent activations
            # xg layout: [128, 8 gate-chunks * TOK], col = mu*TOK + t*bc + b
            # one buffer, reused fwd then bwd (Tile WAR deps serialize)
            xg = {d: pp.tile([128, 8 * TOK], FP8, tag=f"xg{d}", name=f"xg{d}")
                  for d in "fb"}
            # h layout: [128, 2 hid-chunks * TOK], col = k*TOK + t*bc + b
            hT = {d: pp.tile([128, 2 * TOK], BF16, tag=f"h{d}", name=f"h{d}")
                  for d in "fb"}

            with tc.tile_pool(name="xt", bufs=1) as xtp, \
                 tc.tile_pool(name="xrp", bufs=2) as xrp, \
                 tc.tile_pool(name="rec", bufs=1) as recp, \
                 tc.tile_pool(name="st", bufs=3) as stp, \
                 tc.tile_pool(name="fc", bufs=1) as fcp, \
                 tc.tile_pool(name="emo", bufs=2) as emop, \
                 tc.tile_pool(name="ps0", bufs=2, space="PSUM") as ps0p, \
                 tc.tile_pool(name="ps2", bufs=2, space="PSUM") as ps2p:
                # ---------- phase 0: gather + transpose -> xT ----------
                xT = [xtp.tile([rn, TOK], BF16, tag=f"xT{ki}", name=f"xT{ki}")
                      for ki, (r0, rn) in enumerate(ek)]
                idx_all = xtp.tile([128, n_ttile], I32, tag="idx_all")
                nc.gpsimd.dma_start(
                    idx_all[:],
                    toks[:].rearrange("(i p) one -> p (i one)", p=128))
                GC = min(8, n_ttile)        # token tiles per gather chunk
                if "0" not in parts:
                    GC = 0
                for c0 in range(0, n_ttile, GC if GC else n_ttile + 1):
                    xr = xrp.tile([128, GC * EP], F32, tag="xr", name="xr")
                    for j in range(GC):
                        i = c0 + j
                        if "D" in parts:
                            nc.gpsimd.dma_start(
                                out=xr[:, j * EP:(j + 1) * EP],
                                in_=emb_aug[i * 128:(i + 1) * 128, :])
                        else:
                            nc.gpsimd.indirect_dma_start(
                                out=xr[:, j * EP:(j + 1) * EP], out_offset=None,
                                in_=emb_aug[:],
                                in_offset=bass.IndirectOffsetOnAxis(
                                    ap=idx_all[:, i:i + 1], axis=0),
                            )
                    for j in range(GC):
                        i = c0 + j
                        for ki, (r0, rn) in enumerate(ek):
                            pt = ps0p.tile([128, 128], F32, tag="tp")
                            nc.tensor.transpose(
                                out=pt[:rn, :],
                                in_=xr[:, j * EP + r0:j * EP + r0 + rn],
                                identity=id_sb[:])
                            nc.vector.tensor_copy(
                                out=xT[ki][:, i * 128:(i + 1) * 128],
                                in_=pt[:rn, :])

                c_st = {d: recp.tile([128, 2 * bc], F32, tag=f"c{d}",
                                     name=f"c{d}") for d in "fb"}
                for d in "fb":
                    # ------- phase 1: xg = wx^T @ xT (bias via ones-row) ----
                    for mu in range(8):
                        for n in range(n_n512):
                            ps = ps0p.tile([128, 512], F32, tag="mm")
                            for ki, (r0, rn) in enumerate(ek):
                                nc.tensor.matmul(
                                    ps[:],
                                    lhsT=wx_sb[(d, ki)][:, mu * 128:(mu + 1) * 128],
                                    rhs=xT[ki][:, n * 512:(n + 1) * 512],
                                    start=(ki == 0), stop=(ki == 2))
                            nc.scalar.copy(
                                out=xg[d][:, mu * TOK + n * 512:
                                          mu * TOK + (n + 1) * 512],
                                in_=ps[:])

                # -------- phase 2: recurrence, fwd+bwd interleaved ----------
                # gate chunk order is i,f,o,g (host permutes weights):
                # sigmoid on [0:6bc], tanh on [6bc:8bc]
                for step in range(t_steps):
                    for d in "fb":
                        t = step if d == "f" else t_steps - 1 - step
                        first = step == 0
                        gp = stp.tile([128, 8 * bc], F32, tag=f"gp{d}",
                                      name=f"gp{d}")
                        xga = xg[d][:].rearrange(
                            "p (m t) -> p m t", m=8)[:, :, t * bc:(t + 1) * bc]
                        if first:
                            nc.vector.tensor_copy(out=gp[:].rearrange(
                                "p (m c) -> p m c", m=8), in_=xga)
                        else:
                            tprev = t - 1 if d == "f" else t + 1
                            ps = ps2p.tile([128, 8 * bc], F32, tag=f"ps{d}",
                                           name=f"ps{d}")
                            for mu in range(8):
                                for ki in range(2):
                                    nc.tensor.matmul(
                                        ps[:, mu * bc:(mu + 1) * bc],
                                        lhsT=wh_sb[(d, ki)][:, mu * 128:(mu + 1) * 128],
                                        rhs=hT[d][:, ki * TOK + tprev * bc:
                                                  ki * TOK + (tprev + 1) * bc],
                                        start=(ki == 0), stop=(ki == 1))
                            nc.vector.tensor_tensor(
                                out=gp[:].rearrange("p (m c) -> p m c", m=8),
                                in0=ps[:].rearrange("p (m c) -> p m c", m=8),
                                in1=xga, op=ALU.add)
                        sa = stp.tile([128, 8 * bc], F32, tag=f"sa{d}",
                                      name=f"sa{d}")
                        nc.scalar.activation(sa[:, 0:6 * bc], gp[:, 0:6 * bc],
                                             AF.Sigmoid)
                        nc.scalar.activation(sa[:, 6 * bc:8 * bc],
                                             gp[:, 6 * bc:8 * bc], AF.Tanh)
                        t1 = stp.tile([128, 2 * bc], F32, tag=f"t1{d}",
                                      name=f"t1{d}")
                        t2 = stp.tile([128, 2 * bc], F32, tag=f"t2{d}",
                                      name=f"t2{d}")
                        if first:
                            nc.vector.tensor_tensor(
                                out=c_st[d][:], in0=sa[:, 0:2 * bc],
                                in1=sa[:, 6 * bc:8 * bc], op=ALU.mult)
                        else:
                            nc.vector.tensor_tensor(
                                out=t1[:], in0=sa[:, 2 * bc:4 * bc],
                                in1=c_st[d][:], op=ALU.mult)
                            nc.vector.tensor_tensor(
                                out=t2[:], in0=sa[:, 0:2 * bc],
                                in1=sa[:, 6 * bc:8 * bc], op=ALU.mult)
                            nc.vector.tensor_tensor(
                                out=c_st[d][:], in0=t1[:], in1=t2[:],
                                op=ALU.add)
                        tcl = stp.tile([128, 2 * bc], F32, tag=f"tc{d}",
                                       name=f"tc{d}")
                        nc.scalar.activation(tcl[:], c_st[d][:], AF.Tanh)
                        hdst = hT[d][:].rearrange(
                            "p (k t) -> p k t", k=2)[:, :, t * bc:(t + 1) * bc]
                        nc.vector.tensor_tensor(
                            out=hdst,
                            in0=sa[:, 4 * bc:6 * bc].rearrange(
                                "p (k c) -> p k c", k=2),
                            in1=tcl[:].rearrange("p (k c) -> p k c", k=2),
                            op=ALU.mult)

                # ---------- phase 3: fc1 + relu, fc2 + bias, out ----------
                z = fcp.tile([32, TOK], BF16, tag="z")
                if "f" not in parts:
                    n_n512_f = 0
                else:
                    n_n512_f = n_n512
                for n in range(n_n512_f):
                    ps = ps0p.tile([32, 512], F32, tag="mm")
                    for ki in range(4):
                        dd = "f" if ki < 2 else "b"
                        kk = ki % 2
                        nc.tensor.matmul(
                            ps[:], lhsT=fc1w_sb[ki],
                            rhs=hT[dd][:, kk * TOK + n * 512:kk * TOK + (n + 1) * 512],
                            start=(ki == 0), stop=(ki == 3))
                    nc.scalar.activation(z[:, n * 512:(n + 1) * 512], ps[:],
                                         AF.Relu, bias=fc1b_sb[:, :1])
                for n in range(n_n512_f):
                    ps = ps0p.tile([K, 512], F32, tag="mm")
                    nc.tensor.matmul(ps[:], lhsT=fc2w_sb[:],
                                     rhs=z[:, n * 512:(n + 1) * 512],
                                     start=True, stop=True)
                    em = emop.tile([K, 512], F32, tag="em", name="em")
                    nc.vector.tensor_scalar_add(em[:], ps[:], fc2b_sb[:, :1])
                    nc.sync.dma_start(out[:, n * 512:(n + 1) * 512], em[:])
    nc.compile()
    return nc


def _prep_shared(emb, w_ih_f, w_hh_f, b_ih_f, b_hh_f, w_ih_b, w_hh_b,
                 b_ih_b, b_hh_b, fc1_w, fc1_b, fc2_w, fc2_b):
    f32 = np.float32
    emb_aug = np.zeros((V, EP), f32)
    emb_aug[:, :E] = np.asarray(emb, f32)
    emb_aug[0, :E] = 0.0
    emb_aug[:, E] = 1.0

    perm = np.r_[0:512, 768:1024, 512:768]  # i,f,g,o -> i,f,o,g

    def wx(w_ih, b_ih, b_hh):
        m = np.zeros((EP, G4H), f32)
        m[:E, :] = np.asarray(w_ih, f32).T
        m[E, :] = np.asarray(b_ih, f32) + np.asarray(b_hh, f32)
        return m[:, perm].astype(bfloat16).copy()

    return dict(
        emb_aug=emb_aug,
        wxf=wx(w_ih_f, b_ih_f, b_hh_f),
        wxb=wx(w_ih_b, b_ih_b, b_hh_b),
        whf=np.asarray(w_hh_f, np.float32).T[:, perm].astype(bfloat16).copy(),
        whb=np.asarray(w_hh_b, np.float32).T[:, perm].astype(bfloat16).copy(),
        fc1w=np.asarray(fc1_w, np.float32).T.astype(bfloat16).copy(),
        fc1b=np.asarray(fc1_b, np.float32).reshape(32, 1).copy(),
        fc2w=np.asarray(fc2_w, np.float32).T.astype(bfloat16).copy(),
        fc2b=np.asarray(fc2_b, np.float32).reshape(K, 1).copy(),
        iden=np.eye(128, dtype=np.float32),
    )


def _crf_host(emis, tags, mask, start_trans, trans, end_trans):
    # emis: [T, B, K] f32; exact forward algorithm in float64 on host
    emis = emis.astype(np.float64)
    trans = np.asarray(trans, np.float64)
    start = np.asarray(start_trans, np.float64)
    end = np.asarray(end_trans, np.float64)
    tags = np.asarray(tags, np.int64)
    m = np.asarray(mask, np.float64).T           # [T, B]
    tg = tags.T                                  # [T, B]
    Bsz = emis.shape[1]
    bidx = np.arange(Bsz)

    score = start[tg[0]] + emis[0, bidx, tg[0]]
    for t in range(1, emis.shape[0]):
        score = score + (trans[tg[t - 1], tg[t]] + emis[t, bidx, tg[t]]) * m[t]
    seq_ends = np.asarray(mask, np.int64).sum(1) - 1
    score = score + end[tg[seq_ends, bidx]]

    alpha = start[None, :] + emis[0]
    for t in range(1, emis.shape[0]):
        nxt = alpha[:, :, None] + trans[None] + emis[t][:, None, :]
        mx = nxt.max(axis=1)
        nxt = mx + np.log(np.exp(nxt - mx[:, None, :]).sum(axis=1))
        alpha = np.where(m[t][:, None] > 0, nxt, alpha)
    av = alpha + end[None, :]
    mx = av.max(axis=1)
    logZ = mx + np.log(np.exp(av - mx[:, None]).sum(axis=1))
    return -(score - logZ).mean()


_CACHE = {}


def _make_runner():
    import jax
    from jax.sharding import Mesh, PartitionSpec, NamedSharding
    try:
        from jax.experimental.shard_map import shard_map
    except ImportError:
        from jax import shard_map
    from concourse import bass2jax
    from concourse.bass2jax import _bass_exec_p, partition_id_tensor

    nc = build_bass()
    bass2jax.install_neuronx_cc_hook()
    partition_name = (nc.partition_id_tensor.name
                      if nc.partition_id_tensor else None)
    in_names, out_names, out_avals, zero_outs = [], [], [], []
    for alloc in nc.m.functions[0].allocations:
        if not isinstance(alloc, mybir.MemoryLocationSet):
            continue
        name = alloc.memorylocations[0].name
        if alloc.kind == "ExternalInput":
            if name != partition_name:
                in_names.append(name)
        elif alloc.kind == "ExternalOutput":
            shape = tuple(alloc.tensor_shape)
            dtype = mybir.dt.np(alloc.dtype)
            out_names.append(name)
            out_avals.append(jax.core.ShapedArray(shape, dtype))
            zero_outs.append(np.zeros(shape, dtype))
    n_params = len(in_names)
    in_names_all = in_names + out_names
    if partition_name is not None:
        in_names_all.append(partition_name)

    def _body(*args):
        operands = list(args)
        if partition_name is not None:
            operands.append(partition_id_tensor())
        return tuple(_bass_exec_p.bind(
            *operands, out_avals=tuple(out_avals),
            in_names=tuple(in_names_all), out_names=tuple(out_names),
            lowering_input_output_aliases=(),
            sim_require_finite=True, sim_require_nnan=True, nc=nc))

    devices = jax.devices()[:NCORES]
    mesh = Mesh(np.asarray(devices), ("core",))
    donate = tuple(range(n_params, n_params + len(out_names)))
    sharded = jax.jit(
        shard_map(_body, mesh=mesh,
                  in_specs=(PartitionSpec("core"),) * (n_params + len(out_names)),
                  out_specs=(PartitionSpec("core"),) * len(out_names),
                  check_rep=False),
        donate_argnums=donate, keep_unused=True)
    sh = NamedSharding(mesh, PartitionSpec("core"))
    return dict(jax=jax, sharded=sharded, sh=sh, in_names=in_names,
                out_names=out_names, zero_outs=zero_outs)


def _run_device(in_maps):
    if "rt" not in _CACHE:
        _CACHE["rt"] = _make_runner()
    rt = _CACHE["rt"]
    jax = rt["jax"]
    concat_in = [np.concatenate([np.asarray(m[n]) for m in in_maps], 0)
                 for n in rt["in_names"]]
    rt["dev_in"] = [jax.device_put(a, rt["sh"]) for a in concat_in]
    return _exec(rt)


def _exec(rt):
    jax = rt["jax"]
    zo = [jax.device_put(np.concatenate([z] * NCORES, 0), rt["sh"])
          for z in rt["zero_outs"]]
    outs = rt["sharded"](*rt["dev_in"], *zo)
    jax.block_until_ready(outs)
    e = np.asarray(outs[0])            # [NCORES*K, TOK]
    return [e[c * K:(c + 1) * K] for c in range(NCORES)]


def kernel_rerun():
    return _exec(_CACHE["rt"])


def kernel(emb, w_ih_f, w_hh_f, b_ih_f, b_hh_f, w_ih_b, w_hh_b, b_ih_b,
           b_hh_b, fc1_w, fc1_b, fc2_w, fc2_b, start_trans, trans, end_trans,
           tokens, tags, mask):
    shared = _prep_shared(emb, w_ih_f, w_hh_f, b_ih_f, b_hh_f, w_ih_b,
                          w_hh_b, b_ih_b, b_hh_b, fc1_w, fc1_b, fc2_w, fc2_b)
    tokens = np.asarray(tokens)
    in_maps = []
    for c in range(NCORES):
        tk = tokens[c * BC:(c + 1) * BC, :].astype(np.int32)  # [BC, T]
        tk = tk.T.reshape(T * BC, 1).copy()                   # t-major
        in_maps.append({**shared, "toks": tk})

    core_emis = _run_device(in_maps)

    emis = np.zeros((T, B, K), np.float32)
    for c in range(NCORES):
        e = np.asarray(core_emis[c])                          # [K, T*BC]
        emis[:, c * BC:(c + 1) * BC, :] = (
            e.reshape(K, T, BC).transpose(1, 2, 0))
    loss = _crf_host(emis, tags, mask, start_trans, trans, end_trans)
    return np.float32(loss)



# revision 2
# speedup vs baseline: 46.7307x; 46.7307x over previous
"""BiLSTM-CRF forward loss on 8 Trainium2 NeuronCores.

Data-parallel: batch 64 -> 8 sequences per core. Each core runs
embedding gather -> BiLSTM(T=512,H=256) -> fc1(32)+relu -> fc2(4),
and outputs its emissions [4, T*8]. The tiny CRF dynamic program
(O(T*B*K^2), K=4) and the final mean run on host in numpy.
"""

import sys
for _p in ("/opt/trn_rl_repo", "/root/.axon_site/_ro/trn_rl_repo"):
    if _p not in sys.path:
        sys.path.insert(0, _p)
import numpy as np
from ml_dtypes import bfloat16

import concourse.bass as bass
import concourse.bacc as bacc
import concourse.mybir as mybir
from concourse.tile import TileContext
from concourse import bass_utils

B, T, E, H, V, K = 64, 512, 300, 256, 50000, 4
NCORES = 8
BC = B // NCORES          # 8 sequences per core
EP = 304                  # E padded to 304; row 300 = ones (bias trick)
G4H = 4 * H               # 1024
F32 = mybir.dt.float32
BF16 = mybir.dt.bfloat16
I32 = mybir.dt.int32
FP8 = mybir.dt.float8e4
AF = mybir.ActivationFunctionType
ALU = mybir.AluOpType


def build_bass(t_steps=T, bc=BC, parts="012f"):
    TOK = t_steps * bc
    nc = bacc.Bacc()

    # ---- DRAM parameters ----
    emb_aug = nc.dram_tensor("emb_aug", [V, EP], F32, kind="ExternalInput")
    toks = nc.dram_tensor("toks", [TOK, 1], I32, kind="ExternalInput")
    wxf = nc.dram_tensor("wxf", [EP, G4H], BF16, kind="ExternalInput")
    wxb = nc.dram_tensor("wxb", [EP, G4H], BF16, kind="ExternalInput")
    whf = nc.dram_tensor("whf", [H, G4H], BF16, kind="ExternalInput")
    whb = nc.dram_tensor("whb", [H, G4H], BF16, kind="ExternalInput")
    fc1w = nc.dram_tensor("fc1w", [2 * H, 32], BF16, kind="ExternalInput")
    fc1b = nc.dram_tensor("fc1b", [32, 1], F32, kind="ExternalInput")
    fc2w = nc.dram_tensor("fc2w", [32, K], BF16, kind="ExternalInput")
    fc2b = nc.dram_tensor("fc2b", [K, 1], F32, kind="ExternalInput")
    iden = nc.dram_tensor("iden", [128, 128], F32, kind="ExternalInput")
    out = nc.dram_tensor("out", [K, TOK], F32, kind="ExternalOutput")

    n_ttile = TOK // 128          # token tiles of 128
    n_n512 = TOK // 512           # 512-wide token chunks
    ek = [(0, 128), (128, 128), (256, 48)]   # E-chunks (rows of EP)

    with TileContext(nc) as tc:
        with tc.tile_pool(name="const", bufs=1) as constp, \
             tc.tile_pool(name="persist", bufs=1) as pp:
            # constants in SBUF
            id_sb = constp.tile([128, 128], F32, tag="iden")
            nc.sync.dma_start(id_sb[:], iden[:])
            wx_sb = {}
            for d, src in (("f", wxf), ("b", wxb)):
                for ki, (r0, rn) in enumerate(ek):
                    w = constp.tile([rn, G4H], BF16, tag=f"wx{d}{ki}")
                    nc.sync.dma_start(w[:], src[r0:r0 + rn, :])
                    wx_sb[(d, ki)] = w
            wh_sb = {}
            for d, src in (("f", whf), ("b", whb)):
                for ki in range(2):
                    w = constp.tile([128, G4H], BF16, tag=f"wh{d}{ki}")
                    nc.sync.dma_start(w[:], src[ki * 128:(ki + 1) * 128, :])
                    wh_sb[(d, ki)] = w
            fc1w_sb = []
            for ki in range(4):
                w = constp.tile([128, 32], BF16, tag=f"fc1w{ki}")
                nc.sync.dma_start(w[:], fc1w[ki * 128:(ki + 1) * 128, :])
                fc1w_sb.append(w)
            fc2w_sb = constp.tile([32, K], BF16, tag="fc2w")
            nc.sync.dma_start(fc2w_sb[:], fc2w[:])
            fc1b_sb = constp.tile([32, 1], F32, tag="fc1b")
            nc.sync.dma_start(fc1b_sb[:], fc1b[:])
            fc2b_sb = constp.tile([K, 1], F32, tag="fc2b")
            nc.sync.dma_start(fc2b_sb[:], fc2b[:])

            # persistent activations
            # xg layout: [128, 8 gate-chunks * TOK], col = mu*TOK + t*bc + b
            # one buffer, reused fwd then bwd (Tile WAR deps serialize)
            xg = {d: pp.tile([128, 8 * TOK], FP8, tag=f"xg{d}", name=f"xg{d}")
                  for d in "fb"}
            # h layout: [128, 2 hid-chunks * TOK], col = k*TOK + t*bc + b
            hT = {d: pp.tile([128, 2 * TOK], BF16, tag=f"h{d}", name=f"h{d}")
                  for d in "fb"}

            with tc.tile_pool(name="xt", bufs=1) as xtp, \
                 tc.tile_pool(name="xrp", bufs=2) as xrp, \
                 tc.tile_pool(name="rec", bufs=1) as recp, \
                 tc.tile_pool(name="st", bufs=3) as stp, \
                 tc.tile_pool(name="fc", bufs=1) as fcp, \
                 tc.tile_pool(name="emo", bufs=2) as emop, \
                 tc.tile_pool(name="ps0", bufs=2, space="PSUM") as ps0p, \
                 tc.tile_pool(name="ps2", bufs=2, space="PSUM") as ps2p:
                # ---------- phase 0: gather + transpose -> xT ----------
                xT = [xtp.tile([rn, TOK], BF16, tag=f"xT{ki}", name=f"xT{ki}")
                      for ki, (r0, rn) in enumerate(ek)]
                idx_all = xtp.tile([128, n_ttile], I32, tag="idx_all")
                nc.gpsimd.dma_start(
                    idx_all[:],
                    toks[:].rearrange("(i p) one -> p (i one)", p=128))
                GC = min(8, n_ttile)        # token tiles per gather chunk
                if "0" not in parts:
                    GC = 0
                for c0 in range(0, n_ttile, GC if GC else n_ttile + 1):
                    xr = xrp.tile([128, GC * EP], F32, tag="xr", name="xr")
                    for j in range(GC):
                        i = c0 + j
                        if "D" in parts:
                            nc.gpsimd.dma_start(
                                out=xr[:, j * EP:(j + 1) * EP],
                                in_=emb_aug[i * 128:(i + 1) * 128, :])
                        else:
                            nc.gpsimd.indirect_dma_start(
                                out=xr[:, j * EP:(j + 1) * EP], out_offset=None,
                                in_=emb_aug[:],
                                in_offset=bass.IndirectOffsetOnAxis(
                                    ap=idx_all[:, i:i + 1], axis=0),
                            )
                    for j in range(GC):
                        i = c0 + j
                        for ki, (r0, rn) in enumerate(ek):
                            pt = ps0p.tile([128, 128], F32, tag="tp")
                            nc.tensor.transpose(
                                out=pt[:rn, :],
                                in_=xr[:, j * EP + r0:j * EP + r0 + rn],
                                identity=id_sb[:])
                            nc.vector.tensor_copy(
                                out=xT[ki][:, i * 128:(i + 1) * 128],
                                in_=pt[:rn, :])

                c_st = {d: recp.tile([128, 2 * bc], F32, tag=f"c{d}",
                                     name=f"c{d}") for d in "fb"}
                for d in "fb":
                    # ------- phase 1: xg = wx^T @ xT (bias via ones-row) ----
                    for mu in range(8):
                        for n in range(n_n512):
                            ps = ps0p.tile([128, 512], F32, tag="mm")
                            for ki, (r0, rn) in enumerate(ek):
                                nc.tensor.matmul(
                                    ps[:],
                                    lhsT=wx_sb[(d, ki)][:, mu * 128:(mu + 1) * 128],
                                    rhs=xT[ki][:, n * 512:(n + 1) * 512],
                                    start=(ki == 0), stop=(ki == 2))
                            nc.scalar.copy(
                                out=xg[d][:, mu * TOK + n * 512:
                                          mu * TOK + (n + 1) * 512],
                                in_=ps[:])

                # -------- phase 2: recurrence, fwd+bwd interleaved ----------
                # gate chunk order is i,f,o,g (host permutes weights):
                # sigmoid on [0:6bc], tanh on [6bc:8bc]
                for step in range(t_steps):
                    for d in "fb":
                        t = step if d == "f" else t_steps - 1 - step
                        first = step == 0
                        gp = stp.tile([128, 8 * bc], F32, tag=f"gp{d}",
                                      name=f"gp{d}")
                        xga = xg[d][:].rearrange(
                            "p (m t) -> p m t", m=8)[:, :, t * bc:(t + 1) * bc]
                        if first:
                            nc.vector.tensor_copy(out=gp[:].rearrange(
                                "p (m c) -> p m c", m=8), in_=xga)
                        else:
                            tprev = t - 1 if d == "f" else t + 1
                            ps = ps2p.tile([128, 8 * bc], F32, tag=f"ps{d}",
                                           name=f"ps{d}")
                            for mu in range(8):
                                for ki in range(2):
                                    nc.tensor.matmul(
                                        ps[:, mu * bc:(mu + 1) * bc],
                                        lhsT=wh_sb[(d, ki)][:, mu * 128:(mu + 1) * 128],
                                        rhs=hT[d][:, ki * TOK + tprev * bc:
                                                  ki * TOK + (tprev + 1) * bc],
                                        start=(ki == 0), stop=(ki == 1))
                            nc.vector.tensor_tensor(
                                out=gp[:].rearrange("p (m c) -> p m c", m=8),
                                in0=ps[:].rearrange("p (m c) -> p m c", m=8),
                                in1=xga, op=ALU.add)
                        sa = stp.tile([128, 8 * bc], F32, tag=f"sa{d}",
                                      name=f"sa{d}")
                        nc.scalar.activation(sa[:, 0:6 * bc], gp[:, 0:6 * bc],
                                             AF.Sigmoid)
                        nc.scalar.activation(sa[:, 6 * bc:8 * bc],
                                             gp[:, 6 * bc:8 * bc], AF.Tanh)
                        t1 = stp.tile([128, 2 * bc], F32, tag=f"t1{d}",
                                      name=f"t1{d}")
                        t2 = stp.tile([128, 2 * bc], F32, tag=f"t2{d}",
                                      name=f"t2{d}")
                        if first:
                            nc.vector.tensor_tensor(
                                out=c_st[d][:], in0=sa[:, 0:2 * bc],
                                in1=sa[:, 6 * bc:8 * bc], op=ALU.mult)
                        else:
                            nc.vector.tensor_tensor(
                                out=t1[:], in0=sa[:, 2 * bc:4 * bc],
                                in1=c_st[d][:], op=ALU.mult)
                            nc.vector.tensor_tensor(
                                out=t2[:], in0=sa[:, 0:2 * bc],
                                in1=sa[:, 6 * bc:8 * bc], op=ALU.mult)
                            nc.vector.tensor_tensor(
                                out=c_st[d][:], in0=t1[:], in1=t2[:],
                                op=ALU.add)
                        tcl = stp.tile([128, 2 * bc], F32, tag=f"tc{d}",
                                       name=f"tc{d}")
                        nc.scalar.activation(tcl[:], c_st[d][:], AF.Tanh)
                        hdst = hT[d][:].rearrange(
                            "p (k t) -> p k t", k=2)[:, :, t * bc:(t + 1) * bc]
                        nc.vector.tensor_tensor(
                            out=hdst,
                            in0=sa[:, 4 * bc:6 * bc].rearrange(
                                "p (k c) -> p k c", k=2),
                            in1=tcl[:].rearrange("p (k c) -> p k c", k=2),
                            op=ALU.mult)

                # ---------- phase 3: fc1 + relu, fc2 + bias, out ----------
                z = fcp.tile([32, TOK], BF16, tag="z")
                if "f" not in parts:
                    n_n512_f = 0
                else:
                    n_n512_f = n_n512
                for n in range(n_n512_f):
                    ps = ps0p.tile([32, 512], F32, tag="mm")
                    for ki in range(4):
                        dd = "f" if ki < 2 else "b"
                        kk = ki % 2
                        nc.tensor.matmul(
                            ps[:], lhsT=fc1w_sb[ki],
                            rhs=hT[dd][:, kk * TOK + n * 512:kk * TOK + (n + 1) * 512],
                            start=(ki == 0), stop=(ki == 3))
                    nc.scalar.activation(z[:, n * 512:(n + 1) * 512], ps[:],
                                         AF.Relu, bias=fc1b_sb[:, :1])
                for n in range(n_n512_f):
                    ps = ps0p.tile([K, 512], F32, tag="mm")
                    nc.tensor.matmul(ps[:], lhsT=fc2w_sb[:],
                                     rhs=z[:, n * 512:(n + 1) * 512],
                                     start=True, stop=True)
                    em = emop.tile([K, 512], F32, tag="em", name="em")
                    nc.vector.tensor_scalar_add(em[:], ps[:], fc2b_sb[:, :1])
                    nc.sync.dma_start(out[:, n * 512:(n + 1) * 512], em[:])
    nc.compile()
    return nc


def _prep_shared(emb, w_ih_f, w_hh_f, b_ih_f, b_hh_f, w_ih_b, w_hh_b,
                 b_ih_b, b_hh_b, fc1_w, fc1_b, fc2_w, fc2_b):
    f32 = np.float32
    emb_aug = np.zeros((V, EP), f32)
    emb_aug[:, :E] = np.asarray(emb, f32)
    emb_aug[0, :E] = 0.0
    emb_aug[:, E] = 1.0

    perm = np.r_[0:512, 768:1024, 512:768]  # i,f,g,o -> i,f,o,g

    def wx(w_ih, b_ih, b_hh):
        m = np.zeros((EP, G4H), f32)
        m[:E, :] = np.asarray(w_ih, f32).T
        m[E, :] = np.asarray(b_ih, f32) + np.asarray(b_hh, f32)
        return m[:, perm].astype(bfloat16).copy()

    return dict(
        emb_aug=emb_aug,
        wxf=wx(w_ih_f, b_ih_f, b_hh_f),
        wxb=wx(w_ih_b, b_ih_b, b_hh_b),
        whf=np.asarray(w_hh_f, np.float32).T[:, perm].astype(bfloat16).copy(),
        whb=np.asarray(w_hh_b, np.float32).T[:, perm].astype(bfloat16).copy(),
        fc1w=np.asarray(fc1_w, np.float32).T.astype(bfloat16).copy(),
        fc1b=np.asarray(fc1_b, np.float32).reshape(32, 1).copy(),
        fc2w=np.asarray(fc2_w, np.float32).T.astype(bfloat16).copy(),
        fc2b=np.asarray(fc2_b, np.float32).reshape(K, 1).copy(),
        iden=np.eye(128, dtype=np.float32),
    )


def _crf_host(emis, tags, mask, start_trans, trans, end_trans):
    # emis: [T, B, K] f32; exact forward algorithm in float64 on host
    emis = emis.astype(np.float64)
    trans = np.asarray(trans, np.float64)
    start = np.asarray(start_trans, np.float64)
    end = np.asarray(end_trans, np.float64)
    tags = np.asarray(tags, np.int64)
    m = np.asarray(mask, np.float64).T           # [T, B]
    tg = tags.T                                  # [T, B]
    Bsz = emis.shape[1]
    bidx = np.arange(Bsz)

    score = start[tg[0]] + emis[0, bidx, tg[0]]
    for t in range(1, emis.shape[0]):
        score = score + (trans[tg[t - 1], tg[t]] + emis[t, bidx, tg[t]]) * m[t]
    seq_ends = np.asarray(mask, np.int64).sum(1) - 1
    score = score + end[tg[seq_ends, bidx]]

    alpha = start[None, :] + emis[0]
    for t in range(1, emis.shape[0]):
        nxt = alpha[:, :, None] + trans[None] + emis[t][:, None, :]
        mx = nxt.max(axis=1)
        nxt = mx + np.log(np.exp(nxt - mx[:, None, :]).sum(axis=1))
        alpha = np.where(m[t][:, None] > 0, nxt, alpha)
    av = alpha + end[None, :]
    mx = av.max(axis=1)
    logZ = mx + np.log(np.exp(av - mx[:, None]).sum(axis=1))
    return -(score - logZ).mean()


_CACHE = {}


def _make_runner():
    import jax
    from jax.sharding import Mesh, PartitionSpec, NamedSharding
    try:
        from jax.experimental.shard_map import shard_map
    except ImportError:
        from jax import shard_map
    from concourse import bass2jax
    from concourse.bass2jax import _bass_exec_p, partition_id_tensor

    nc = build_bass()
    bass2jax.install_neuronx_cc_hook()
    partition_name = (nc.partition_id_tensor.name
                      if nc.partition_id_tensor else None)
    in_names, out_names, out_avals, zero_outs = [], [], [], []
    for alloc in nc.m.functions[0].allocations:
        if not isinstance(alloc, mybir.MemoryLocationSet):
            continue
        name = alloc.memorylocations[0].name
        if alloc.kind == "ExternalInput":
            if name != partition_name:
                in_names.append(name)
        elif alloc.kind == "ExternalOutput":
            shape = tuple(alloc.tensor_shape)
            dtype = mybir.dt.np(alloc.dtype)
            out_names.append(name)
            out_avals.append(jax.core.ShapedArray(shape, dtype))
            zero_outs.append(np.zeros(shape, dtype))
    n_params = len(in_names)
    in_names_all = in_names + out_names
    if partition_name is not None:
        in_names_all.append(partition_name)

    def _body(*args):
        operands = list(args)
        if partition_name is not None:
            operands.append(partition_id_tensor())
        return tuple(_bass_exec_p.bind(
            *operands, out_avals=tuple(out_avals),
            in_names=tuple(in_names_all), out_names=tuple(out_names),
            lowering_input_output_aliases=(),
            sim_require_finite=True, sim_require_nnan=True, nc=nc))

    devices = jax.devices()[:NCORES]
    mesh = Mesh(np.asarray(devices), ("core",))
    sh = NamedSharding(mesh, PartitionSpec("core"))
    # The kernel writes every element of its outputs, so the zero output
    # buffers are NOT donated: they are uploaded once and reused by every
    # execution (saves one ~70ms host->device sync per run).
    sm = shard_map(_body, mesh=mesh,
                   in_specs=(PartitionSpec("core"),) * (n_params + len(out_names)),
                   out_specs=(PartitionSpec("core"),) * len(out_names),
                   check_rep=False)
    return dict(jax=jax, sm=sm, sh=sh, in_names=in_names,
                out_names=out_names, zero_outs=zero_outs)


def _run_device(in_maps):
    if "rt" not in _CACHE:
        _CACHE["rt"] = _make_runner()
    rt = _CACHE["rt"]
    jax = rt["jax"]
    from concourse.bass2jax import fast_dispatch_compile
    concat_in = [np.concatenate([np.asarray(m[n]) for m in in_maps], 0)
                 for n in rt["in_names"]]
    rt["dev_in"] = [jax.device_put(a, rt["sh"]) for a in concat_in]
    rt["zo_dev"] = [jax.device_put(np.concatenate([z] * NCORES, 0), rt["sh"])
                    for z in rt["zero_outs"]]
    if "sharded" not in rt:
        args = tuple(rt["dev_in"]) + tuple(rt["zo_dev"])
        try:
            rt["sharded"] = fast_dispatch_compile(
                lambda: jax.jit(rt["sm"], keep_unused=True)
                .lower(*args).compile())
        except Exception:
            rt["sharded"] = jax.jit(rt["sm"], keep_unused=True)
    return _exec(rt)


def _exec(rt):
    outs = rt["sharded"](*rt["dev_in"], *rt["zo_dev"])
    e = np.asarray(outs[0])            # [NCORES*K, TOK]; blocks until done
    return [e[c * K:(c + 1) * K] for c in range(NCORES)]


def kernel_rerun(n=1):
    """Execute the compiled kernel n times back-to-back (one sync at the
    end) and return the last run's per-core outputs."""
    rt = _CACHE["rt"]
    outs = None
    for _ in range(n):
        outs = rt["sharded"](*rt["dev_in"], *rt["zo_dev"])
    e = np.asarray(outs[0])
    return [e[c * K:(c + 1) * K] for c in range(NCORES)]


def kernel(emb, w_ih_f, w_hh_f, b_ih_f, b_hh_f, w_ih_b, w_hh_b, b_ih_b,
           b_hh_b, fc1_w, fc1_b, fc2_w, fc2_b, start_trans, trans, end_trans,
           tokens, tags, mask):
    shared = _prep_shared(emb, w_ih_f, w_hh_f, b_ih_f, b_hh_f, w_ih_b,
                          w_hh_b, b_ih_b, b_hh_b, fc1_w, fc1_b, fc2_w, fc2_b)
    tokens = np.asarray(tokens)
    in_maps = []
    for c in range(NCORES):
        tk = tokens[c * BC:(c + 1) * BC, :].astype(np.int32)  # [BC, T]
        tk = tk.T.reshape(T * BC, 1).copy()                   # t-major
        in_maps.append({**shared, "toks": tk})

    core_emis = _run_device(in_maps)

    emis = np.zeros((T, B, K), np.float32)
    for c in range(NCORES):
        e = np.asarray(core_emis[c])                          # [K, T*BC]
        emis[:, c * BC:(c + 1) * BC, :] = (
            e.reshape(K, T, BC).transpose(1, 2, 0))
    loss = _crf_host(emis, tags, mask, start_trans, trans, end_trans)
    return np.float32(loss)

